# revision 1
# baseline (speedup 1.0000x reference)
"""Trainium2 Bass kernel for the CAM factorized-attention module.

Reference computation (per batch element b, C=256, N=P*H*W=12288, h=8 heads,
Ch=32):
    x1   = x[b].reshape(C, N).T                      # [N, C]
    qkv  = x1 @ W_qkv + b_qkv                        # [N, 3C]
    q, k, v  (each [h, N, Ch])
    kw   = softmax(k, axis=N)
    kv   = kw^T @ v (per head)                       # [h, Ch, Ch]
    fa   = q @ kv                                    # [h, N, Ch]
    out  = (scale * fa).reshape(N, C) @ W_proj + b_proj
    res  = gamma * out.T.reshape(C, P, H, W) + x[b]

Sharding: data-parallel over B — core i computes batch element i, no
collectives.

Precision plan: the attention branch is ~0.3% of the output magnitude
(output = x + gamma*attn with |gamma*attn| tiny), so the branch tolerates
aggressive quantization.  The two large matmul families (k/v projection and
the final collapsed M @ x) run in fp8e4 DoubleRow mode (2 MACs/cell/cycle,
contraction 256 in one pass); everything downstream of the softmax
(kv, fold) runs bf16 with fp32 PSUM accumulation; the residual x is added in
exact fp32.  End-to-end error vs the fp32 reference is ~2e-5 (CPU-verified).

Algebraic restructuring (exact up to rounding):
  * k bias cancels in softmax (constant along the softmax axis)  -> dropped.
  * no max-subtraction needed (|k| < ~4); the softmax denominator is applied
    to the tiny per-head [Ch, Ch] kv matrix, not the [N, C] weight field.
    Denominators come free as an extra ones column in the kv matmul.
  * v bias folds into kv:  kv_true = (E^T v_raw)/S + b_v (row vec).
  * scale & gamma fold into W_proj;  gamma folds into b_proj (host side).
  * q is never materialized, and once kv is known the whole branch collapses
    to ONE linear map of x:
        attn^T = M^T x + bias_eff 1^T
        M[kc][mt]  = sum_t  Wq[kc,tblk] @ kvblk[t] @ Wp'[tblk,mtblk]
        bias_eff   = sum_t  Wp'[tblk,mtblk]^T kvblk[t]^T bq[tblk] + bp'
    M ([256,256] total) is fused on-chip with 14 tiny matmuls after the kv
    accumulation finishes, scaled by 2^17 into fp8e4 range (entries are
    ~1e-4; the 2^-17 unscale rides the ACT epilogue's scale parameter).

Per-core pipeline:
  load x8 (fp8, [ki,ko,n] with c = ko*128+ki) + wkv8 first; xf (fp32, for
  the exact residual) streams in the background during phase 1
  phase 1 (48 pairs of 128-token chunks):
    k||v = x8^T wkv8  (one DoubleRow matmul per chunk, PSUM [128,1024]/pair)
    E = exp(k) (one ACT op per pair);  vb = [v|1] bf16 (one DVE copy/pair)
    kvps[pi%2] += E_half^T vb_half  (two parity-alternating PSUM tiles so
    consecutive pairs' accumulation matmuls are independent)
  finalize: kvsum = kvps[0]+kvps[1];  kvblk = diag(kvsum)/S + bv  (bf16)
  fold: G' = kvblk^T Wq^T;  M8 = 2^17 * G'^T Wp' (fp8);  bias_eff
  phase 2 (24 chunks of 512):  pp = M8^T x8  (one DoubleRow matmul per mt);
    tmp = pp*2^-17 + bias_eff (ACT);  osb = tmp + xf (DVE);  DMA out.  Phase 2 is bounded by the 12.6 MB output stream (~35 us);
    total DMA (29.2 MB at ~360 GB/s/core) is ~92% utilized end to end.
"""

import sys

sys.path.insert(0, "/opt/trn_rl_repo")

import numpy as np
import ml_dtypes

import concourse.bacc as bacc
import concourse.mybir as mybir
from concourse.tile import TileContext
from concourse.bass_utils import run_bass_kernel_spmd

FP32 = mybir.dt.float32
BF16 = mybir.dt.bfloat16
FP8 = mybir.dt.float8e4
FP16 = mybir.dt.float16
AF = mybir.ActivationFunctionType
DR = mybir.MatmulPerfMode.DoubleRow

C = 256
N = 12288
NCORES = 8
NPAIR = N // 256  # 48 pairs of 128-token chunks
NJUMBO = N // 512  # 24 chunks of 512 tokens
NPIECE = 4  # xf load granularity
M_SCALE = 131072.0  # 2^17

_CACHE = {}


def _build_nc(debug=False):
    from concourse.alu_op_type import AluOpType

    nc = bacc.Bacc(trn_type="TRN2", target_bir_lowering=False)

    x8_d = nc.declare_dram_parameter("x8", [128, 2, N], FP8, False)
    xf_d = nc.declare_dram_parameter("xf", [2, 128, N], FP32, False)
    wkv8_d = nc.declare_dram_parameter("wkv8", [128, 2, 512], FP8, False)
    wqt_d = nc.declare_dram_parameter("wqt", [2, 128, 256], BF16, False)
    wp_d = nc.declare_dram_parameter("wp", [2, 128, 256], BF16, False)
    bq_d = nc.declare_dram_parameter("bq", [2, 128, 1], BF16, False)
    bp_d = nc.declare_dram_parameter("bp", [2, 128, 1], FP32, False)
    bv_d = nc.declare_dram_parameter("bv", [2, 128, 32], FP32, False)
    out_d = nc.declare_dram_parameter("out", [2, 128, N], FP32, True)
    if debug:
        dbg_kvps = nc.declare_dram_parameter("dbg_kvps", [2, 128, 129], FP32, True)
        dbg_kvblk = nc.declare_dram_parameter("dbg_kvblk", [2, 128, 128], BF16, True)
        dbg_be = nc.declare_dram_parameter("dbg_be", [2, 128, 1], FP32, True)

    PIECE = N // NPIECE

    with TileContext(nc) as tc:
        with (
            tc.tile_pool(name="const", bufs=1) as const,
            tc.tile_pool(name="resident", bufs=1) as resident,
        ):
            # --- resident tensors -------------------------------------------
            x8 = resident.tile([128, 2, N], FP8, name="x8")
            xf = [resident.tile([128, N], FP32, name=f"xf{t}") for t in range(2)]
            wkv8 = const.tile([128, 2, 512], FP8, name="wkv8")
            wqt = [const.tile([128, 256], BF16, name=f"wqt{t}") for t in range(2)]
            wp = [const.tile([128, 256], BF16, name=f"wp{t}") for t in range(2)]
            bq = [const.tile([128, 1], BF16, name=f"bq{t}") for t in range(2)]
            bp = [const.tile([128, 1], FP32, name=f"bp{t}") for t in range(2)]
            bv = [const.tile([128, 32], FP32, name=f"bv{t}") for t in range(2)]
            kvblk = [const.tile([128, 128], BF16, name=f"kvblk{t}") for t in range(2)]
            Gp = [
                [const.tile([128, 128], BF16, name=f"Gp{t}{kc}") for kc in range(2)]
                for t in range(2)
            ]
            M8 = [const.tile([128, 2, 128], FP8, name=f"M8{mt}") for mt in range(2)]
            cq = [const.tile([128, 1], BF16, name=f"cq{t}") for t in range(2)]
            be = [const.tile([128, 1], FP32, name=f"be{mt}") for mt in range(2)]
            recip = [const.tile([128, 1], FP32, name=f"recip{t}") for t in range(2)]
            vb = [const.tile([128, 516], BF16, name=f"vb{j}") for j in range(3)]
            kvsum = const.tile([128, 258], FP32, name="kvsum")

            # phase-1 gates first: x8 (piecewise so chunk 0 starts asap) + wkv8
            nc.sync.dma_start(x8[:, :, 0 : N // 8], x8_d[:, :, 0 : N // 8])
            nc.sync.dma_start(wkv8[:], wkv8_d[:, :, :])
            for i in range(1, 8):
                nc.sync.dma_start(
                    x8[:, :, i * N // 8 : (i + 1) * N // 8],
                    x8_d[:, :, i * N // 8 : (i + 1) * N // 8],
                )
            for t in range(2):
                nc.sync.dma_start(wqt[t][:], wqt_d[t])
                nc.sync.dma_start(wp[t][:], wp_d[t])
                nc.sync.dma_start(bq[t][:], bq_d[t])
                nc.sync.dma_start(bp[t][:], bp_d[t])
                nc.sync.dma_start(bv[t][:], bv_d[t])
                nc.vector.memset(kvblk[t][:], 0.0)
            for j in range(3):
                nc.vector.memset(
                    vb[j][:].rearrange("p (s x) -> p s x", x=129)[:, :, 128:129], 1.0
                )
            # xf only matters from phase 2 on; stream it during phase 1
            for i in range(NPIECE):
                for t in range(2):
                    nc.sync.dma_start(
                        xf[t][:, i * PIECE : (i + 1) * PIECE],
                        xf_d[t, :, i * PIECE : (i + 1) * PIECE],
                    )

            # --- phase 1: k||v, exp, kv accumulation ------------------------
            with (
                tc.tile_pool(name="p1ps", bufs=1, space="PSUM") as p1ps,
                tc.tile_pool(name="kvp_ps", bufs=3, space="PSUM") as kvp_ps,
                tc.tile_pool(name="ework", bufs=6) as ework,
            ):
                # two parity-alternating accumulators (t0 at cols 0:129, t1 at
                # 129:258) so consecutive pairs' kv matmuls are independent
                kvps = [
                    p1ps.tile([128, 258], FP32, name=f"kvps{par}") for par in range(2)
                ]

                for pi in range(NPAIR):
                    par = pi % 2
                    first, last = pi < 2, pi >= NPAIR - 2
                    kvp = kvp_ps.tile([128, 1024], FP32, name="kvp", tag="kvp")
                    for half in range(2):
                        n0 = (pi * 2 + half) * 128
                        f0 = half * 512
                        nc.tensor.matmul(
                            kvp[:, f0 : f0 + 512],
                            lhsT=x8[:, :, n0 : n0 + 128], rhs=wkv8[:],
                            start=True, stop=True, perf_mode=DR,
                        )
                    # one exp over both chunks' k columns (strided view)
                    E = ework.tile([128, 512], BF16, name="E", tag="E")
                    nc.scalar.activation(
                        E[:].rearrange("p (s x) -> p s x", x=256),
                        kvp[:].rearrange("p (s x) -> p s x", x=512)[:, :, 0:256],
                        AF.Exp,
                    )
                    v = vb[pi % 3]
                    nc.vector.tensor_copy(
                        v[:].rearrange("p (h t x) -> p h t x", t=2, x=129)[
                            :, :, :, 0:128
                        ],
                        kvp[:]
                        .rearrange("p (h x) -> p h x", x=512)[:, :, 256:512]
                        .rearrange("p h (t c) -> p h t c", c=128),
                    )
                    for half in range(2):
                        for t in range(2):
                            sec = half * 2 + t
                            nc.tensor.matmul(
                                kvps[par][:, t * 129 : t * 129 + 129],
                                lhsT=E[
                                    :,
                                    half * 256 + t * 128 : half * 256 + t * 128 + 128,
                                ],
                                rhs=v[:, sec * 129 : sec * 129 + 129],
                                start=(first and half == 0),
                                stop=(last and half == 1),
                                skip_group_check=True,
                            )

                # --- finalize kv: merge parities, normalize, add v bias -----
                nc.vector.tensor_copy(kvsum[:], kvps[0][:])
                nc.vector.tensor_add(kvsum[:], kvsum[:], kvps[1][:])
                if debug:
                    for t in range(2):
                        nc.sync.dma_start(
                            dbg_kvps[t], kvsum[:, t * 129 : t * 129 + 129]
                        )
                for t in range(2):
                    c0 = t * 129
                    nc.vector.reciprocal(recip[t][:], kvsum[:, c0 + 128 : c0 + 129])
                    for g in range(4):
                        r0 = g * 32
                        nc.vector.scalar_tensor_tensor(
                            kvblk[t][r0 : r0 + 32, r0 : r0 + 32],
                            kvsum[r0 : r0 + 32, c0 + r0 : c0 + r0 + 32],
                            recip[t][r0 : r0 + 32, :],
                            bv[t][r0 : r0 + 32, :],
                            op0=AluOpType.mult,
                            op1=AluOpType.add,
                        )

            # --- fold: G' = kvblk^T Wq^T, M8 = 2^17 G'^T Wp', bias_eff ------
            with tc.tile_pool(name="gps", bufs=4, space="PSUM") as gps:
                for t in range(2):
                    cq_ps = gps.tile([128, 1], FP32, name=f"cqps{t}", tag="little")
                    nc.tensor.matmul(
                        cq_ps[:], lhsT=kvblk[t][:], rhs=bq[t][:],
                        start=True, stop=True,
                    )
                    nc.vector.tensor_copy(cq[t][:], cq_ps[:])
                    for kc in range(2):
                        g_ps = gps.tile([128, 128], FP32, name=f"gps{t}{kc}", tag="big")
                        nc.tensor.matmul(
                            g_ps[:],
                            lhsT=kvblk[t][:],
                            rhs=wqt[t][:, kc * 128 : kc * 128 + 128],
                            start=True, stop=True,
                        )
                        nc.vector.tensor_copy(Gp[t][kc][:], g_ps[:])
                for mt in range(2):
                    be_ps = gps.tile([128, 1], FP32, name=f"beps{mt}", tag="little")
                    for t in range(2):
                        nc.tensor.matmul(
                            be_ps[:],
                            lhsT=wp[t][:, mt * 128 : mt * 128 + 128],
                            rhs=cq[t][:],
                            start=(t == 0), stop=(t == 1),
                        )
                    nc.vector.tensor_add(be[mt][:], be_ps[:], bp[mt][:])
                    for kc in range(2):
                        m_ps = gps.tile([128, 128], FP32, name=f"mps{kc}{mt}", tag="big")
                        for t in range(2):
                            nc.tensor.matmul(
                                m_ps[:],
                                lhsT=Gp[t][kc][:],
                                rhs=wp[t][:, mt * 128 : mt * 128 + 128],
                                start=(t == 0), stop=(t == 1),
                            )
                        nc.scalar.activation(
                            M8[mt][:, kc, :], m_ps[:], AF.Identity, scale=M_SCALE
                        )
                if debug:
                    for mt in range(2):
                        nc.sync.dma_start(dbg_be[mt], be[mt][:])
                    for t in range(2):
                        nc.sync.dma_start(dbg_kvblk[t], kvblk[t][:])

            # --- phase 2: pp = M8^T x8;  out = pp/2^17 + bias_eff + xf ------
            with (
                tc.tile_pool(name="pp_ps", bufs=6, space="PSUM") as pp_ps,
                tc.tile_pool(name="p2out", bufs=8) as p2out,
            ):
                for cj in range(NJUMBO):
                    n0 = cj * 512
                    for mt in range(2):
                        pp = pp_ps.tile([128, 512], FP32, name="pp", tag="pp")
                        nc.tensor.matmul(
                            pp[:], lhsT=M8[mt][:], rhs=x8[:, :, n0 : n0 + 512],
                            start=True, stop=True, perf_mode=DR,
                        )
                        osb = p2out.tile([128, 512], FP32, name="osb", tag="osb")
                        tmp = p2out.tile([128, 512], FP32, name="tmp", tag="tmp")
                        nc.scalar.activation(
                            tmp[:], pp[:], AF.Identity,
                            bias=be[mt][:], scale=1.0 / M_SCALE,
                        )
                        nc.vector.tensor_add(
                            osb[:], tmp[:], xf[mt][:, n0 : n0 + 512]
                        )
                        nc.sync.dma_start(out_d[mt, :, n0 : n0 + 512], osb[:])
    nc.finalize()
    return nc


def _get_nc():
    if "nc" not in _CACHE:
        _CACHE["nc"] = _build_nc()
    return _CACHE["nc"]


def _prep_in_maps(x, W_qkv, b_qkv, W_proj, b_proj, gamma):
    bf = ml_dtypes.bfloat16
    f8 = ml_dtypes.float8_e4m3
    scale = 32 ** (-0.5)
    g = float(np.asarray(gamma).reshape(-1)[0])

    # fp8 operands use contraction index c = ko*128 + ki -> layout [ki, ko, :]
    Wkv8 = np.ascontiguousarray(
        W_qkv[:, 256:768].reshape(2, 128, 512).swapaxes(0, 1)).astype(f8)
    WqT = np.ascontiguousarray(
        W_qkv[:, 0:256].T.reshape(2, 128, 256)).astype(bf)
    Wp = np.ascontiguousarray(
        (W_proj * (scale * g)).reshape(2, 128, 256)).astype(bf)
    bq = np.ascontiguousarray(
        b_qkv[0:256].reshape(2, 128, 1)).astype(bf)
    bp = np.ascontiguousarray(
        (g * b_proj).reshape(2, 128, 1)).astype(np.float32)
    # bv[t][p, cv] = b_qkv[512 + (t*4 + p//32)*32 + cv]
    bv = np.ascontiguousarray(
        np.broadcast_to(
            b_qkv[512:768].reshape(2, 4, 1, 32), (2, 4, 32, 32)
        ).reshape(2, 128, 32)
    ).astype(np.float32)

    in_maps = []
    for b in range(NCORES):
        xb = np.ascontiguousarray(x[b].reshape(C, N))
        x8 = np.ascontiguousarray(
            xb.reshape(2, 128, N).swapaxes(0, 1)).astype(f8)
        in_maps.append(
            {
                "x8": x8,
                "xf": xb.reshape(2, 128, N),
                "wkv8": Wkv8, "wqt": WqT, "wp": Wp,
                "bq": bq, "bp": bp, "bv": bv,
            }
        )
    return in_maps


def kernel(x, W_qkv, b_qkv, W_proj, b_proj, gamma, _trace=False, _trace_kwargs=None):
    x = np.asarray(x, dtype=np.float32)
    nc = _get_nc()
    in_maps = _prep_in_maps(
        x,
        np.asarray(W_qkv, np.float32),
        np.asarray(b_qkv, np.float32),
        np.asarray(W_proj, np.float32),
        np.asarray(b_proj, np.float32),
        np.asarray(gamma, np.float32),
    )
    kw = {}
    if _trace:
        kw = {"trace": True, **(_trace_kwargs or {})}
    res = run_bass_kernel_spmd(nc, in_maps, list(range(NCORES)), **kw)
    out = np.stack(
        [res.results[b]["out"].reshape(C, 3, 64, 64) for b in range(NCORES)]
    ).astype(np.float32)
    if _trace:
        return out, res
    return out



# revision 5
# speedup vs baseline: 1.0463x; 1.0463x over previous
"""Trainium2 Bass kernel for the CAM factorized-attention module.

Reference computation (per batch element b, C=256, N=P*H*W=12288, h=8 heads,
Ch=32):
    x1   = x[b].reshape(C, N).T                      # [N, C]
    qkv  = x1 @ W_qkv + b_qkv                        # [N, 3C]
    q, k, v  (each [h, N, Ch])
    kw   = softmax(k, axis=N)
    kv   = kw^T @ v (per head)                       # [h, Ch, Ch]
    fa   = q @ kv                                    # [h, N, Ch]
    out  = (scale * fa).reshape(N, C) @ W_proj + b_proj
    res  = gamma * out.T.reshape(C, P, H, W) + x[b]

Sharding: data-parallel over B — core i computes batch element i, no
collectives.

Precision plan: the attention branch is ~0.3% of the output magnitude
(output = x + gamma*attn with |gamma*attn| tiny), so the branch tolerates
aggressive quantization; the residual path needs only bf16 (output rel err
~2e-3 vs the 2e-2 gate, host-verified in numpy).  x ships once as bf16
(residual) and once as fp8e4 (matmul operand); all big matmuls run fp8
DoubleRow (contraction 256 in one pass, 2 cols/cycle); E=exp(k) and v are
stored fp8e4 so the kv accumulation is DoubleRow too.  The folded map M is
cast to fp8e5m2 at NATURAL scale (entries ~1e-4 sit in e5m2 normal range),
which removes the 2^17 descale and lets the whole phase-2 epilogue collapse
to a single scalar_tensor_tensor per tile: out = (pp + be) + xbf.

Algebraic restructuring (exact up to rounding):
  * k bias cancels in softmax -> dropped; no max-subtraction needed (|k|<~5).
  * softmax denominators ride as a ones column in the kv matmul; the
    normalization is applied to the tiny per-head [Ch,Ch] kv matrix.
  * v bias folds into kv; scale & gamma fold into W_proj; gamma into b_proj.
  * the whole branch collapses to ONE linear map: out = M^T x + be 1^T + x,
    M = Wq kvblk Wp' fused on-chip with 14 tiny matmuls after phase 1.

Engine budget per core (cost-model): ACT = exp only (~29us), DVE/Pool split
the v-copies (phase 1) and the STT drains (phase 2), PE ~45.6k cycles, DMA
~16MB at 360GB/s (~44.5us total, split ~27 in / ~17.5 out across the two
phases).  Phase 1 is exp-bound (~30us), phase 2 out-DMA-bound (~19us).
"""

import sys

sys.path.insert(0, "/opt/trn_rl_repo")

import numpy as np
import ml_dtypes

import concourse.bacc as bacc
import concourse.mybir as mybir
from concourse.tile import TileContext
from concourse.bass_utils import run_bass_kernel_spmd

FP32 = mybir.dt.float32
BF16 = mybir.dt.bfloat16
FP8 = mybir.dt.float8e4
FP8E5 = mybir.dt.float8e5
AF = mybir.ActivationFunctionType
DR = mybir.MatmulPerfMode.DoubleRow

C = 256
N = 12288
NCORES = 8
NPAIR = N // 256   # 48 pairs of 128-token chunks
NJUMBO = N // 512  # 24 phase-2 chunks of 512 tokens
NSLAB = 6          # output DMA slabs of 2048 tokens
VD = 5             # of every 8 v-copies, this many go to DVE (rest Pool)
DD = 7             # of every 12 phase-2 drains, this many go to DVE

_CACHE = {}


def _build_nc():
    from concourse.alu_op_type import AluOpType

    nc = bacc.Bacc(trn_type="TRN2", target_bir_lowering=False)

    x8_d = nc.declare_dram_parameter("x8", [128, 2, N], FP8, False)
    xbf_d = nc.declare_dram_parameter("xbf", [2, 128, N], BF16, False)
    wkv8_d = nc.declare_dram_parameter("wkv8", [128, 2, 512], FP8, False)
    # bf16 pack: cols 0:256 WqT, 256:512 Wp', 512:513 bq
    wbf_d = nc.declare_dram_parameter("wbf", [2, 128, 513], BF16, False)
    # fp32 pack: col 0 bp', cols 1:33 bv
    wf32_d = nc.declare_dram_parameter("wf32", [2, 128, 33], FP32, False)
    out_d = nc.declare_dram_parameter("out", [128, 2, N], BF16, True)

    with TileContext(nc) as tc:
        with (
            tc.tile_pool(name="const", bufs=1) as const,
            tc.tile_pool(name="resident", bufs=1) as resident,
        ):
            # --- resident tensors -------------------------------------------
            x8 = resident.tile([128, 2, N], FP8, name="x8")
            xbf = [resident.tile([128, N], BF16, name=f"xbf{t}") for t in range(2)]
            osl = [resident.tile([128, 2, 2048], BF16, name=f"osl{s}") for s in range(2)]
            wkv8 = const.tile([128, 2, 512], FP8, name="wkv8")
            wbf = [const.tile([128, 513], BF16, name=f"wbf{t}") for t in range(2)]
            wf32 = [const.tile([128, 33], FP32, name=f"wf32{t}") for t in range(2)]
            kvblk = [const.tile([128, 128], BF16, name=f"kvblk{t}") for t in range(2)]
            Gp = [
                [const.tile([128, 128], BF16, name=f"Gp{t}{kc}") for kc in range(2)]
                for t in range(2)
            ]
            M8 = [const.tile([128, 2, 128], FP8E5, name=f"M8{mt}") for mt in range(2)]
            cq = [const.tile([128, 1], BF16, name=f"cq{t}") for t in range(2)]
            be = [const.tile([128, 1], FP32, name=f"be{mt}") for mt in range(2)]
            recip = [const.tile([128, 1], FP32, name=f"recip{t}") for t in range(2)]
            vb = [const.tile([128, 516], FP8, name=f"vb{j}") for j in range(3)]
            kvsum = const.tile([128, 258], FP32, name="kvsum")

            # phase-1 gates first: x8 (piecewise so chunk 0 starts asap) + wkv8
            nc.sync.dma_start(x8[:, :, 0 : N // 8], x8_d[:, :, 0 : N // 8])
            nc.sync.dma_start(wkv8[:], wkv8_d[:, :, :])
            for i in range(1, 8):
                nc.sync.dma_start(
                    x8[:, :, i * N // 8 : (i + 1) * N // 8],
                    x8_d[:, :, i * N // 8 : (i + 1) * N // 8],
                )
            for t in range(2):
                nc.sync.dma_start(wbf[t][:], wbf_d[t])
                nc.sync.dma_start(wf32[t][:], wf32_d[t])
                nc.vector.memset(kvblk[t][:], 0.0)
            for j in range(3):
                nc.vector.memset(
                    vb[j][:].rearrange("p (s x) -> p s x", x=129)[:, :, 128:129], 1.0
                )
            # xbf only matters from phase 2 on; stream it during phase 1
            PIECE = N // 4
            for i in range(4):
                for t in range(2):
                    nc.sync.dma_start(
                        xbf[t][:, i * PIECE : (i + 1) * PIECE],
                        xbf_d[t, :, i * PIECE : (i + 1) * PIECE],
                    )

            wqt = [wbf[t][:, 0:256] for t in range(2)]
            wp = [wbf[t][:, 256:512] for t in range(2)]
            bq = [wbf[t][:, 512:513] for t in range(2)]
            bp = [wf32[t][:, 0:1] for t in range(2)]
            bv = [wf32[t][:, 1:33] for t in range(2)]

            # --- phase 1: k||v, exp, fp8 kv accumulation (DoubleRow) --------
            with (
                tc.tile_pool(name="p1ps", bufs=1, space="PSUM") as p1ps,
                tc.tile_pool(name="kvp_ps", bufs=3, space="PSUM") as kvp_ps,
                tc.tile_pool(name="ework", bufs=6) as ework,
            ):
                kvps = [
                    p1ps.tile([128, 258], FP32, name=f"kvps{par}") for par in range(2)
                ]

                # emit kv matmuls with lag 2 so the in-order PE never waits
                # on the exp/v-copy of the pair it just produced
                pending = []

                def emit_kv(pi, E, v):
                    par = pi % 2
                    first, last = pi < 2, pi >= NPAIR - 2
                    Ev = E[:].rearrange("p (s x) -> p s x", x=256)
                    vv = v[:].rearrange("p (s t x) -> p s t x", t=2, x=129)
                    for t in range(2):
                        nc.tensor.matmul(
                            kvps[par][:, t * 129 : t * 129 + 129],
                            lhsT=Ev[:, :, t * 128 : t * 128 + 128],
                            rhs=vv[:, :, t, :],
                            start=first, stop=last,
                            perf_mode=DR, skip_group_check=True,
                        )

                for pi in range(NPAIR):
                    kvp = kvp_ps.tile([128, 1024], FP32, name="kvp", tag="kvp")
                    for half in range(2):
                        n0 = (pi * 2 + half) * 128
                        f0 = half * 512
                        nc.tensor.matmul(
                            kvp[:, f0 : f0 + 512],
                            lhsT=x8[:, :, n0 : n0 + 128], rhs=wkv8[:],
                            start=True, stop=True, perf_mode=DR,
                        )
                    if len(pending) >= 2:
                        emit_kv(*pending.pop(0))
                    # one exp over both chunks' k columns (strided view)
                    E = ework.tile([128, 512], FP8, name="E", tag="E")
                    nc.scalar.activation(
                        E[:].rearrange("p (s x) -> p s x", x=256),
                        kvp[:].rearrange("p (s x) -> p s x", x=512)[:, :, 0:256],
                        AF.Exp,
                    )
                    v = vb[pi % 3]
                    nc.vector.tensor_copy(
                        v[:].rearrange("p (h t x) -> p h t x", t=2, x=129)[
                            :, :, :, 0:128
                        ],
                        kvp[:]
                        .rearrange("p (h x) -> p h x", x=512)[:, :, 256:512]
                        .rearrange("p h (t c) -> p h t c", c=128),
                    )
                    pending.append((pi, E, v))
                while pending:
                    emit_kv(*pending.pop(0))

                # --- finalize kv: merge parities, normalize, add v bias -----
                nc.vector.tensor_copy(kvsum[:], kvps[0][:])
                nc.vector.tensor_add(kvsum[:], kvsum[:], kvps[1][:])
                for t in range(2):
                    c0 = t * 129
                    nc.vector.reciprocal(recip[t][:], kvsum[:, c0 + 128 : c0 + 129])
                    for g in range(4):
                        r0 = g * 32
                        nc.vector.scalar_tensor_tensor(
                            kvblk[t][r0 : r0 + 32, r0 : r0 + 32],
                            kvsum[r0 : r0 + 32, c0 + r0 : c0 + r0 + 32],
                            recip[t][r0 : r0 + 32, :],
                            bv[t][r0 : r0 + 32, :],
                            op0=AluOpType.mult,
                            op1=AluOpType.add,
                        )

            # --- fold: G' = kvblk^T Wq^T, M8 = G'^T Wp' (fp8e5), bias_eff ---
            with tc.tile_pool(name="gps", bufs=4, space="PSUM") as gps:
                for t in range(2):
                    cq_ps = gps.tile([128, 1], FP32, name=f"cqps{t}", tag="little")
                    nc.tensor.matmul(
                        cq_ps[:], lhsT=kvblk[t][:], rhs=bq[t],
                        start=True, stop=True,
                    )
                    nc.vector.tensor_copy(cq[t][:], cq_ps[:])
                    for kc in range(2):
                        g_ps = gps.tile([128, 128], FP32, name=f"gps{t}{kc}", tag="big")
                        nc.tensor.matmul(
                            g_ps[:],
                            lhsT=kvblk[t][:],
                            rhs=wqt[t][:, kc * 128 : kc * 128 + 128],
                            start=True, stop=True,
                        )
                        nc.vector.tensor_copy(Gp[t][kc][:], g_ps[:])
                for mt in range(2):
                    be_ps = gps.tile([128, 1], FP32, name=f"beps{mt}", tag="little")
                    for t in range(2):
                        nc.tensor.matmul(
                            be_ps[:],
                            lhsT=wp[t][:, mt * 128 : mt * 128 + 128],
                            rhs=cq[t][:],
                            start=(t == 0), stop=(t == 1),
                        )
                    nc.vector.tensor_add(be[mt][:], be_ps[:], bp[mt])
                    for kc in range(2):
                        m_ps = gps.tile([128, 128], FP32, name=f"mps{kc}{mt}", tag="big")
                        for t in range(2):
                            nc.tensor.matmul(
                                m_ps[:],
                                lhsT=Gp[t][kc][:],
                                rhs=wp[t][:, mt * 128 : mt * 128 + 128],
                                start=(t == 0), stop=(t == 1),
                            )
                        nc.scalar.activation(M8[mt][:, kc, :], m_ps[:], AF.Identity)

            # --- phase 2: pp = M8^T x8;  out = (pp + be) + xbf ---------------
            # GPSIMD cannot read PSUM, so the drain mixes three engine routes:
            #   (a) DVE solo: one STT  osb = (pp + be) + xbf
            #   (b) ACT bias-pass (pp+be -> bf16 tmp), then DVE add of xbf
            #   (c) ACT bias-pass, then Pool SBUF-only add (via STT w/ 0 imm)
            with (
                tc.tile_pool(name="pp_ps", bufs=8, space="PSUM") as pp_ps,
                tc.tile_pool(name="tmpw", bufs=4) as tmpw,
            ):
                for cj in range(NJUMBO):
                    n0 = cj * 512
                    slab = osl[(cj // 4) % 2]
                    c0 = (cj % 4) * 512
                    for mt in range(2):
                        pp = pp_ps.tile([128, 512], FP32, name="pp", tag="pp")
                        nc.tensor.matmul(
                            pp[:], lhsT=M8[mt][:], rhs=x8[:, :, n0 : n0 + 512],
                            start=True, stop=True, perf_mode=DR,
                        )
                        u = (cj * 2 + mt) % 24
                        if u < 11:
                            nc.vector.scalar_tensor_tensor(
                                slab[:, mt, c0 : c0 + 512],
                                pp[:],
                                be[mt][:],
                                xbf[mt][:, n0 : n0 + 512],
                                op0=AluOpType.add,
                                op1=AluOpType.add,
                            )
                        else:
                            tmp = tmpw.tile([128, 512], BF16, name="tmp", tag="tmp")
                            nc.scalar.activation(
                                tmp[:], pp[:], AF.Identity, bias=be[mt][:]
                            )
                            if u < 20:
                                nc.vector.tensor_add(
                                    slab[:, mt, c0 : c0 + 512],
                                    tmp[:],
                                    xbf[mt][:, n0 : n0 + 512],
                                )
                            else:
                                nc.gpsimd.tensor_add(
                                    slab[:, mt, c0 : c0 + 512],
                                    tmp[:],
                                    xbf[mt][:, n0 : n0 + 512],
                                )
                    if cj % 4 == 3:
                        ns = (cj // 4) * 2048
                        nc.sync.dma_start(
                            out_d[:, :, ns : ns + 2048], slab[:]
                        )
    nc.finalize()
    return nc


def _get_nc():
    if "nc" not in _CACHE:
        _CACHE["nc"] = _build_nc()
    return _CACHE["nc"]


def _prep_in_maps(x, W_qkv, b_qkv, W_proj, b_proj, gamma):
    bf = ml_dtypes.bfloat16
    f8 = ml_dtypes.float8_e4m3
    scale = 32 ** (-0.5)
    g = float(np.asarray(gamma).reshape(-1)[0])

    # fp8 operands use contraction index c = ko*128 + ki -> layout [ki, ko, :]
    Wkv8 = np.ascontiguousarray(
        W_qkv[:, 256:768].reshape(2, 128, 512).swapaxes(0, 1)).astype(f8)
    WqT = W_qkv[:, 0:256].T.reshape(2, 128, 256)
    Wp = (W_proj * (scale * g)).reshape(2, 128, 256)
    bq = b_qkv[0:256].reshape(2, 128, 1)
    wbf = np.ascontiguousarray(
        np.concatenate([WqT, Wp, bq], axis=2)).astype(bf)
    bp = (g * b_proj).reshape(2, 128, 1)
    # bv[t][p, cv] = b_qkv[512 + (t*4 + p//32)*32 + cv]
    bv = np.broadcast_to(
        b_qkv[512:768].reshape(2, 4, 1, 32), (2, 4, 32, 32)
    ).reshape(2, 128, 32)
    wf32 = np.ascontiguousarray(
        np.concatenate([bp, bv], axis=2)).astype(np.float32)

    in_maps = []
    for b in range(NCORES):
        xb = np.ascontiguousarray(x[b].reshape(C, N))
        xbf = xb.reshape(2, 128, N).astype(bf)
        x8 = np.ascontiguousarray(
            xbf.astype(f8).swapaxes(0, 1))
        in_maps.append(
            {
                "x8": x8,
                "xbf": xbf,
                "wkv8": Wkv8, "wbf": wbf, "wf32": wf32,
            }
        )
    return in_maps


def kernel(x, W_qkv, b_qkv, W_proj, b_proj, gamma, _trace=False, _trace_kwargs=None):
    x = np.asarray(x, dtype=np.float32)
    nc = _get_nc()
    in_maps = _prep_in_maps(
        x,
        np.asarray(W_qkv, np.float32),
        np.asarray(b_qkv, np.float32),
        np.asarray(W_proj, np.float32),
        np.asarray(b_proj, np.float32),
        np.asarray(gamma, np.float32),
    )
    kw = {}
    if _trace:
        kw = {"trace": True, **(_trace_kwargs or {})}
    res = run_bass_kernel_spmd(nc, in_maps, list(range(NCORES)), **kw)
    out = np.stack(
        [
            res.results[b]["out"]
            .astype(np.float32)
            .transpose(1, 0, 2)
            .reshape(C, 3, 64, 64)
            for b in range(NCORES)
        ]
    )
    if _trace:
        return out, res
    return out


# revision 14
# speedup vs baseline: 1.3498x; 1.2901x over previous
"""Trainium2 Bass kernel for the CAM factorized-attention module.

Reference computation (per batch element b, C=256, N=P*H*W=12288, h=8 heads,
Ch=32):
    x1   = x[b].reshape(C, N).T                      # [N, C]
    qkv  = x1 @ W_qkv + b_qkv                        # [N, 3C]
    q, k, v  (each [h, N, Ch])
    kw   = softmax(k, axis=N)
    kv   = kw^T @ v (per head)                       # [h, Ch, Ch]
    fa   = q @ kv                                    # [h, N, Ch]
    out  = (scale * fa).reshape(N, C) @ W_proj + b_proj
    res  = gamma * out.T.reshape(C, P, H, W) + x[b]

Sharding: data-parallel over B — core i computes batch element i, no
collectives.

Precision plan: the attention branch is ~0.3% of the output magnitude, so it
tolerates aggressive quantization; the residual path needs only bf16 (output
rel err ~3.8e-3 vs the 2e-2 gate, verified both in numpy and on the device).
x ships once as bf16 (residual) and once as fp8e4 (matmul operand); all big
matmuls run fp8 DoubleRow (contraction 256 in one pass, 2 cols/cycle);
E=exp(k) and v are stored fp8e4 so the kv accumulation is DoubleRow too.
The folded map M is cast to fp8e5m2 at NATURAL scale (entries ~1e-4 sit in
e5m2 normal range), which removes the 2^17 descale so the phase-2 epilogue
is a single op per tile.

Algebraic restructuring (exact up to rounding):
  * k bias cancels in softmax -> dropped; no max-subtraction needed (|k|<~5).
  * softmax denominators ride as a ones column in the kv matmul; the
    normalization is applied to the tiny per-head [Ch,Ch] kv matrix.
  * v bias folds into kv; scale & gamma fold into W_proj; gamma into b_proj.
  * the branch collapses to ONE linear map: out = M^T x + be 1^T + x,
    M = Wq kvblk Wp' fused on-chip with 14 tiny matmuls after phase 1.

Schedule (cost-model): phase 1 iterates 32 groups of 3 chunks (384 tokens):
3 DoubleRow matmuls into a 3-bank PSUM slot (2 slots), one batched exp (ACT)
and one batched v-copy (DVE) per group into resident 6-slot fp8 rings, kv
DoubleRow matmuls lagged 2 groups so the in-order PE never stalls; the kv
accumulator is a single PSUM bank.  Phase 2 alternates two drain routes per
[128,512] tile: even units add the residual INSIDE PSUM via an identity-
matmul accumulate (PE) so the drain is one ACT bias-pass; odd units use one
DVE scalar_tensor_tensor (pp+be)+xbf.  Output leaves in 4 bf16 slabs, one
DMA per 1024 tokens.  Phase 1 is v-copy/exp-bound (~30us), phase 2 is
out-DMA-bound (~19us).
"""

import sys

sys.path.insert(0, "/opt/trn_rl_repo")

import numpy as np
import ml_dtypes

import concourse.bacc as bacc
import concourse.mybir as mybir
from concourse.tile import TileContext
from concourse.bass_utils import run_bass_kernel_spmd

FP32 = mybir.dt.float32
BF16 = mybir.dt.bfloat16
FP8 = mybir.dt.float8e4
FP8E5 = mybir.dt.float8e5
AF = mybir.ActivationFunctionType
DR = mybir.MatmulPerfMode.DoubleRow

C = 256
N = 12288
NCORES = 8
NCHUNK = N // 128   # 96 chunks of 128 tokens
NGROUP = NCHUNK // 3  # 32 phase-1 groups of 3 chunks
NPAIR = NCHUNK // 2   # 48 DoubleRow token-pairs
NJUMBO = N // 512     # 24 phase-2 chunks of 512 tokens

_CACHE = {}


def _build_nc():
    from concourse.alu_op_type import AluOpType

    nc = bacc.Bacc(trn_type="TRN2", target_bir_lowering=False)

    x8_d = nc.declare_dram_parameter("x8", [128, 2, N], FP8, False)
    xbf_d = nc.declare_dram_parameter("xbf", [2, 128, N], BF16, False)
    wkv8_d = nc.declare_dram_parameter("wkv8", [128, 2, 512], FP8, False)
    # bf16 pack: cols 0:256 WqT, 256:512 Wp', 512:513 bq, 513:641 I128,
    # 641:897 M_const (host-folded bv contribution to M, per kc=t: 2 mt blocks)
    wbf_d = nc.declare_dram_parameter("wbf", [2, 128, 897], BF16, False)
    # fp32 pack: col 0 bp', cols 1:33 bv
    wf32_d = nc.declare_dram_parameter("wf32", [2, 128, 33], FP32, False)
    out_d = nc.declare_dram_parameter("out", [128, 2, N], BF16, True)

    with TileContext(nc) as tc:
        with (
            tc.tile_pool(name="const", bufs=1) as const,
            tc.tile_pool(name="resident", bufs=1) as resident,
        ):
            # --- resident tensors -------------------------------------------
            x8 = resident.tile([128, 2, N], FP8, name="x8")
            xbf = [resident.tile([128, N], BF16, name=f"xbf{t}") for t in range(2)]
            osl = [resident.tile([128, 2, 1024], BF16, name=f"osl{s}") for s in range(4)]
            # manual ring of 3 E/v tiles (one per 3-chunk group): separate
            # tile objects so the per-tile dependency tracking pipelines
            E3 = [resident.tile([128, 512], FP8, name=f"E3_{j}") for j in range(8)]
            vb3 = [resident.tile([128, 516], FP8, name=f"vb3_{j}") for j in range(8)]
            wkv8 = const.tile([128, 2, 512], FP8, name="wkv8")
            wbf = [const.tile([128, 897], BF16, name=f"wbf{t}") for t in range(2)]
            wf32 = [const.tile([128, 33], FP32, name=f"wf32{t}") for t in range(2)]
            kvsb = const.tile([128, 256], BF16, name="kvsb")
            wqts = [const.tile([128, 256], BF16, name=f"wqts{t}") for t in range(2)]
            bqs = [const.tile([128, 1], BF16, name=f"bqs{t}") for t in range(2)]
            Gp = [
                [const.tile([128, 128], BF16, name=f"Gp{t}{kc}") for kc in range(2)]
                for t in range(2)
            ]
            M8 = [const.tile([128, 2, 128], FP8E5, name=f"M8{mt}") for mt in range(2)]
            cq = [const.tile([128, 1], BF16, name=f"cq{t}") for t in range(2)]
            be = [const.tile([128, 1], FP32, name=f"be{mt}") for mt in range(2)]
            recip = [const.tile([128, 1], FP32, name=f"recip{t}") for t in range(2)]

            # phase-1 gates first: wkv8, then x8 in staggered pieces so the
            # first matmul starts as early as possible
            n0 = 0
            for i, sz in enumerate((256, 256, 512, 1024, 1536, 2048, 3072, 3584)):
                nc.sync.dma_start(x8[:, :, n0 : n0 + sz], x8_d[:, :, n0 : n0 + sz])
                if i == 0:
                    # wkv8 rides the ACT queue so its HWDGE prep overlaps
                    nc.scalar.dma_start(wkv8[:], wkv8_d[:, :, :])
                n0 += sz
            assert n0 == N
            for t in range(2):
                nc.sync.dma_start(wbf[t][:], wbf_d[t])
                nc.sync.dma_start(wf32[t][:], wf32_d[t])
            # ones columns for the softmax denominators
            for j in range(8):
                nc.vector.memset(
                    vb3[j][:].rearrange("p (s t x) -> p s t x", t=2, x=129)[
                        :, :, :, 128:129
                    ],
                    1.0,
                )
            # xbf only matters from phase 2 on; stream it during phase 1
            PIECE = N // 4
            for i in range(4):
                for t in range(2):
                    nc.sync.dma_start(
                        xbf[t][:, i * PIECE : (i + 1) * PIECE],
                        xbf_d[t, :, i * PIECE : (i + 1) * PIECE],
                    )

            wqt = [wbf[t][:, 0:256] for t in range(2)]
            wp = [wbf[t][:, 256:512] for t in range(2)]
            bq = [wbf[t][:, 512:513] for t in range(2)]
            I128 = wbf[0][:, 513:641]
            Mc = [
                [wbf[kc][:, 641 + mt * 128 : 641 + (mt + 1) * 128] for mt in range(2)]
                for kc in range(2)
            ]
            bp = [wf32[t][:, 0:1] for t in range(2)]
            bv = [wf32[t][:, 1:33] for t in range(2)]

            # --- phase 1: k||v, exp, fp8 kv accumulation (DoubleRow) --------
            # 1-pair (256-token) PSUM slots, 3 buffers: the WAR slack
            # (p1-matmul waits the v-copy 3 pairs back) is ~3x the serial
            # dependency loop, so the DVE v-copies run back-to-back.
            with (
                tc.tile_pool(name="p1ps", bufs=1, space="PSUM") as p1ps,
                tc.tile_pool(name="kvp_ps", bufs=3, space="PSUM") as kvp_ps,
            ):
                kvps = p1ps.tile([128, 258], FP32, name="kvps")

                def emit_kv(pi):
                    Ev = E3[pi % 8][:].rearrange("p (s x) -> p s x", x=256)
                    vv = vb3[pi % 8][:].rearrange("p (s t x) -> p s t x", t=2, x=129)
                    for t in range(2):
                        nc.tensor.matmul(
                            kvps[:, t * 129 : t * 129 + 129],
                            lhsT=Ev[:, :, t * 128 : t * 128 + 128],
                            rhs=vv[:, :, t, :],
                            start=(pi == 0), stop=(pi == NPAIR - 1),
                            perf_mode=DR, skip_group_check=True,
                        )

                for pi in range(NPAIR):
                    kvp = kvp_ps.tile([128, 1024], FP32, name="kvp", tag="kvp")
                    for half in range(2):
                        n0 = (pi * 2 + half) * 128
                        nc.tensor.matmul(
                            kvp[:, half * 512 : half * 512 + 512],
                            lhsT=x8[:, :, n0 : n0 + 128], rhs=wkv8[:],
                            start=True, stop=True, perf_mode=DR,
                        )
                    # kv matmuls for the pair finished 3 iterations ago
                    if pi >= 3:
                        emit_kv(pi - 3)
                    kv2 = kvp[:].rearrange("p (s x) -> p s x", x=512)
                    nc.scalar.activation(
                        E3[pi % 8][:].rearrange("p (s x) -> p s x", x=256),
                        kv2[:, :, 0:256],
                        AF.Exp,
                    )
                    nc.vector.tensor_copy(
                        vb3[pi % 8][:].rearrange("p (s t x) -> p s t x", t=2, x=129)[
                            :, :, :, 0:128
                        ],
                        kv2[:, :, 256:512].rearrange("p s (t c) -> p s t c", c=128),
                    )
                for pi in range(NPAIR - 3, NPAIR):
                    emit_kv(pi)

                # --- finalize: recip, raw-kv diag copy, fold recip into
                # scaled copies of WqT/bq (the bv term was folded on host) ---
                nc.vector.reciprocal(recip[0][:], kvps[:, 128:129])
                nc.vector.tensor_copy(
                    kvsb[:].rearrange("p (t x) -> p t x", x=128),
                    kvps[:].rearrange("p (t x) -> p t x", x=129)[:, :, 0:128],
                )
                nc.vector.reciprocal(recip[1][:], kvps[:, 257:258])
                for t in range(2):
                    nc.vector.tensor_scalar_mul(wqts[t][:], wqt[t], recip[t][:])
                    nc.vector.tensor_scalar_mul(bqs[t][:], bq[t], recip[t][:])

            # --- fold: G' = kvn^T Wq^T, M8 = G'^T Wp' + Mc (fp8e5), bias ---
            # kc-major so each M8 block's inputs finish early; PSUM->SBUF
            # copies split across ACT and DVE to halve the serial chain
            with tc.tile_pool(name="gps", bufs=4, space="PSUM") as gps:
                for kc in range(2):
                    for t in range(2):
                        kvt = kvsb[:, t * 128 : t * 128 + 128]
                        g_ps = gps.tile([128, 128], FP32, name=f"gps{t}{kc}", tag="big")
                        nc.tensor.matmul(
                            g_ps[:],
                            lhsT=kvt,
                            rhs=wqts[t][:, kc * 128 : kc * 128 + 128],
                            start=True, stop=True,
                        )
                        ceng = nc.scalar.copy if t == 0 else nc.vector.tensor_copy
                        ceng(Gp[t][kc][:], g_ps[:])
                for t in range(2):
                    cq_ps = gps.tile([128, 1], FP32, name=f"cqps{t}", tag="little")
                    nc.tensor.matmul(
                        cq_ps[:], lhsT=kvsb[:, t * 128 : t * 128 + 128],
                        rhs=bqs[t][:], start=True, stop=True,
                    )
                    nc.scalar.copy(cq[t][:], cq_ps[:])
                for mt in range(2):
                    for kc in range(2):
                        m_ps = gps.tile([128, 128], FP32, name=f"mps{kc}{mt}", tag="big")
                        for t in range(2):
                            nc.tensor.matmul(
                                m_ps[:],
                                lhsT=Gp[t][kc][:],
                                rhs=wp[t][:, mt * 128 : mt * 128 + 128],
                                start=(t == 0), stop=(t == 1),
                            )
                        nc.vector.tensor_add(M8[mt][:, kc, :], m_ps[:], Mc[kc][mt])
                for mt in range(2):
                    be_ps = gps.tile([128, 1], FP32, name=f"beps{mt}", tag="little")
                    for t in range(2):
                        nc.tensor.matmul(
                            be_ps[:],
                            lhsT=wp[t][:, mt * 128 : mt * 128 + 128],
                            rhs=cq[t][:],
                            start=(t == 0), stop=(t == 1),
                        )
                    nc.vector.tensor_add(be[mt][:], be_ps[:], bp[mt])

            # --- phase 2: pp = M8^T x8 (+ I^T xbf);  drain + be + residual --
            # Two single-pass drain routes per [128,512] tile:
            #   even: residual accumulated in PSUM by an identity matmul,
            #         drain = one ACT bias-pass (pp + be -> bf16)
            #   odd:  one DVE STT  osb = (pp + be) + xbf
            with tc.tile_pool(name="pp_ps", bufs=8, space="PSUM") as pp_ps:
                for cj in range(NJUMBO):
                    n0 = cj * 512
                    slab = osl[(cj // 2) % 4]
                    c0 = (cj % 2) * 512
                    for mt in range(2):
                        act_route = (cj * 2 + mt) % 2 == 0
                        pp = pp_ps.tile([128, 512], FP32, name="pp", tag="pp")
                        nc.tensor.matmul(
                            pp[:], lhsT=M8[mt][:], rhs=x8[:, :, n0 : n0 + 512],
                            start=True, stop=not act_route, perf_mode=DR,
                        )
                        if act_route:
                            nc.tensor.matmul(
                                pp[:], lhsT=I128, rhs=xbf[mt][:, n0 : n0 + 512],
                                start=False, stop=True, skip_group_check=True,
                            )
                            nc.scalar.activation(
                                slab[:, mt, c0 : c0 + 512], pp[:],
                                AF.Identity, bias=be[mt][:],
                            )
                        else:
                            nc.vector.scalar_tensor_tensor(
                                slab[:, mt, c0 : c0 + 512],
                                pp[:],
                                be[mt][:],
                                xbf[mt][:, n0 : n0 + 512],
                                op0=AluOpType.add,
                                op1=AluOpType.add,
                            )
                    if cj == NJUMBO - 2:
                        nc.sync.dma_start(
                            out_d[:, :, cj * 512 : cj * 512 + 512], slab[:, :, 0:512]
                        )
                    elif cj == NJUMBO - 1:
                        nc.sync.dma_start(
                            out_d[:, :, cj * 512 : cj * 512 + 512], slab[:, :, 512:1024]
                        )
                    elif cj % 2 == 1:
                        ns = (cj - 1) * 512
                        nc.sync.dma_start(out_d[:, :, ns : ns + 1024], slab[:])
    nc.finalize()
    return nc


def _get_nc():
    if "nc" not in _CACHE:
        _CACHE["nc"] = _build_nc()
    return _CACHE["nc"]


def _prep_in_maps(x, W_qkv, b_qkv, W_proj, b_proj, gamma):
    bf = ml_dtypes.bfloat16
    f8 = ml_dtypes.float8_e4m3
    scale = 32 ** (-0.5)
    g = float(np.asarray(gamma).reshape(-1)[0])

    # fp8 operands use contraction index c = ko*128 + ki -> layout [ki, ko, :]
    Wkv8 = np.ascontiguousarray(
        W_qkv[:, 256:768].reshape(2, 128, 512).swapaxes(0, 1)).astype(f8)
    Wq = W_qkv[:, 0:256]
    WqT = Wq.T.reshape(2, 128, 256)
    Wpf = W_proj * (scale * g)
    Wp = Wpf.reshape(2, 128, 256)
    bq = b_qkv[0:256].reshape(2, 128, 1)
    I2 = np.broadcast_to(np.eye(128, dtype=np.float32), (2, 128, 128))
    # host-folded bv contribution: Bv[k,v] = bv[v] within each 32-wide head
    bv_vec = b_qkv[512:768]
    head_mask = np.kron(np.eye(8, dtype=np.float32), np.ones((32, 32), np.float32))
    Bv = head_mask * bv_vec[None, :]
    M_const = (Wq @ Bv @ Wpf).astype(np.float32)          # [256, 256]
    Mc = M_const.reshape(2, 128, 2, 128).reshape(2, 128, 256)
    wbf = np.ascontiguousarray(
        np.concatenate([WqT, Wp, bq, I2, Mc], axis=2)).astype(bf)
    bp_eff = (g * b_proj + Wpf.T @ (Bv.T @ b_qkv[0:256])).reshape(2, 128, 1)
    # bv[t][p, cv] = b_qkv[512 + (t*4 + p//32)*32 + cv]
    bv = np.broadcast_to(
        bv_vec.reshape(2, 4, 1, 32), (2, 4, 32, 32)
    ).reshape(2, 128, 32)
    wf32 = np.ascontiguousarray(
        np.concatenate([bp_eff, bv], axis=2)).astype(np.float32)

    in_maps = []
    for b in range(NCORES):
        xb = np.ascontiguousarray(x[b].reshape(C, N))
        xbf = xb.reshape(2, 128, N).astype(bf)
        x8 = np.ascontiguousarray(
            xbf.astype(f8).swapaxes(0, 1))
        in_maps.append(
            {
                "x8": x8,
                "xbf": xbf,
                "wkv8": Wkv8, "wbf": wbf, "wf32": wf32,
            }
        )
    return in_maps


def kernel(x, W_qkv, b_qkv, W_proj, b_proj, gamma, _trace=False, _trace_kwargs=None):
    x = np.asarray(x, dtype=np.float32)
    nc = _get_nc()
    in_maps = _prep_in_maps(
        x,
        np.asarray(W_qkv, np.float32),
        np.asarray(b_qkv, np.float32),
        np.asarray(W_proj, np.float32),
        np.asarray(b_proj, np.float32),
        np.asarray(gamma, np.float32),
    )
    kw = {}
    if _trace:
        kw = {"trace": True, **(_trace_kwargs or {})}
    res = run_bass_kernel_spmd(nc, in_maps, list(range(NCORES)), **kw)
    out = np.stack(
        [
            res.results[b]["out"]
            .astype(np.float32)
            .transpose(1, 0, 2)
            .reshape(C, 3, 64, 64)
            for b in range(NCORES)
        ]
    )
    if _trace:
        return out, res
    return out
